# revision 16
# baseline (speedup 1.0000x reference)
"""Causal self-attention (B=2, T=2048, D=2048, H=16, hd=128) on 8 TRN2 cores.

Sharding: core c = (batch b = c//4, head-group g = c%4).  Each core owns 4
heads (a 512-wide slice of the q/k/v projection outputs and of the out-proj
contraction dim) and one batch.  Each core computes a partial output
(its heads' contribution to x @ wo^T); the host sums the 4 partials per
batch and adds bo.

v2: all matmul operands are fp16 (same 1 cycle/row PE rate as fp32r but
half the DMA bytes and SBUF footprint).  x, and all weights are loaded to
SBUF once.  Q^T/K^T are projected for the FULL sequence per head (4 PSUM
banks = 4x512 columns), V in natural [t, e] layout per 128-row chunk.
Scores are computed transposed ([k, q]) so softmax needs no transposes of
P; the denominator comes from a ones-vector matmul.  exp() runs without
max-subtraction (score range ~N(0, 0.33)).

Causal trim: for the diagonal (same-quarter) key chunks the moving query
range is restricted to the valid suffix, so scores/exp/denominator/AV all
skip the upper triangle at 128-granularity.

PE bubbles in the attention dependency chain (scores -> EXP -> AV) are
plugged with filler matmul groups: the V projection during quarter 0's
attention, the previous quarter's out-projection during later quarters.
"""
import math
from contextlib import ExitStack

import numpy as np

import concourse.bass as bass
import concourse.tile as tile
from concourse import bacc, mybir
from concourse.bass_utils import run_bass_kernel_spmd

D = 2048
T = 2048
B = 2
HD = 128          # head dim
H_PER = 4         # heads per core
ES = 512          # e-slice width per core (H_PER * HD)
NQ = 4            # time quarters
QW = T // NQ      # quarter width (512)
DC = D // 128     # d-chunks (16)
TC = T // 128     # t-chunks (16)
N_CORES = 8

F32 = mybir.dt.float32
F16 = mybir.dt.float16
EXP = mybir.ActivationFunctionType.Exp


def _build():
    nc = bacc.Bacc("TRN2", target_bir_lowering=False, debug=False)
    dram = {}
    for name, shape, dt in [
        ("xT", [D, T], F16),
        ("wqT", [D, ES], F16),
        ("wkT", [D, ES], F16),
        ("wvT", [D, ES], F16),
        ("woT", [ES, D], F16),
        ("bq", [H_PER, 128, 1], F32),
        ("bk", [H_PER, 128, 1], F32),
        ("bv_row", [1, ES], F32),
        ("ones_c", [128, 1], F16),
        ("ones_r", [1, 128], F16),
    ]:
        dram[name] = nc.dram_tensor(name, shape, dt, kind="ExternalInput").ap()
    out_ap = nc.dram_tensor("partial", [T, D], F16, kind="ExternalOutput").ap()

    with tile.TileContext(nc) as tc, ExitStack() as ctx:
        const_p = ctx.enter_context(tc.tile_pool(name="const", bufs=1))
        big_p = ctx.enter_context(tc.tile_pool(name="big", bufs=1))
        e_p = ctx.enter_context(tc.tile_pool(name="expT", bufs=4))
        ctx_p = ctx.enter_context(tc.tile_pool(name="ctxT", bufs=2))
        os_p = ctx.enter_context(tc.tile_pool(name="ostage", bufs=2))
        sm_p = ctx.enter_context(tc.tile_pool(name="small", bufs=2))
        pp_proj = ctx.enter_context(tc.tile_pool(name="pproj", bufs=2, space="PSUM"))
        pp_sc = ctx.enter_context(tc.tile_pool(name="psc", bufs=3, space="PSUM"))
        pp_ctx = ctx.enter_context(tc.tile_pool(name="pctx", bufs=2, space="PSUM"))
        pp_dn = ctx.enter_context(tc.tile_pool(name="pdn", bufs=1, space="PSUM"))

        # ---- constants ----
        bqt = const_p.tile([128, H_PER], F32, tag="bq")
        bkt = const_p.tile([128, H_PER], F32, tag="bk")
        for h in range(H_PER):
            nc.sync.dma_start(out=bqt[:, h : h + 1], in_=dram["bq"][h])
            nc.sync.dma_start(out=bkt[:, h : h + 1], in_=dram["bk"][h])
        ones_c = const_p.tile([128, 1], F16, tag="ones_c")
        nc.sync.dma_start(out=ones_c[:], in_=dram["ones_c"][:])
        ones_r = const_p.tile([1, 128], F16, tag="ones_r")
        nc.sync.dma_start(out=ones_r[:], in_=dram["ones_r"][:])
        bv_row = const_p.tile([1, ES], F32, tag="bv_row")
        nc.sync.dma_start(out=bv_row[:], in_=dram["bv_row"][:])
        bvb = const_p.tile([128, ES], F32, tag="bvb")

        def _bvb_setup():
            # deferred so this chain never blocks the head of the PE queue
            bvr_r = const_p.tile([1, ES], F16, tag="bvr_r")
            nc.vector.tensor_copy(bvr_r[:], bv_row[:])
            pbv = pp_sc.tile([128, ES], F32, tag="sc", name="pbv")
            nc.tensor.matmul(pbv[:], ones_r[:], bvr_r[:], start=True, stop=True)
            nc.scalar.copy(bvb[:], pbv[:])

        # ---- bulk loads: weights (sync queue), x^T (gpsimd queue, streamed
        # quarter-major so quarter-0 projections start early) ----
        # xt[:, dc*T + t] = xT[dc*128 + p, t]
        xt = big_p.tile([128, DC * T], F16, tag="xt")
        for dc in range(DC):   # quarter-0 pieces: fine-grained, land first
            nc.gpsimd.dma_start(
                out=xt[:, dc * T : dc * T + QW],
                in_=dram["xT"][dc * 128 : (dc + 1) * 128, 0:QW],
            )
        for dc in range(DC):   # quarters 1-3: one wide DMA per d-chunk
            nc.gpsimd.dma_start(
                out=xt[:, dc * T + QW : (dc + 1) * T],
                in_=dram["xT"][dc * 128 : (dc + 1) * 128, QW:T],
            )
        wts = {}
        for name in ("wkT", "wqT", "wvT"):
            w = big_p.tile([128, DC * ES], F16, tag=name, name=name + "_t")
            for dc in range(DC):
                nc.sync.dma_start(
                    out=w[:, dc * ES : (dc + 1) * ES],
                    in_=dram[name][dc * 128 : (dc + 1) * 128, :],
                )
            wts[name] = w
        # wot[:, hc*D + eo] = woT[hc*128 + p, eo]
        wot = big_p.tile([128, H_PER * D], F16, tag="wot")
        for hc in range(H_PER):
            nc.sync.dma_start(
                out=wot[:, hc * D : (hc + 1) * D],
                in_=dram["woT"][hc * 128 : (hc + 1) * 128, :],
            )

        # ---- projections, one quarter of t at a time (2-bank sub-passes) ----
        # qt/kt[:, h*T + t] : partition = head-dim
        # Vt[:, tc*ES + e]  : partition = t within chunk tc
        qt = big_p.tile([128, H_PER * T], F16, tag="qt")
        kt = big_p.tile([128, H_PER * T], F16, tag="kt")
        Vt = big_p.tile([128, TC * ES], F16, tag="Vt")

        def _kq_subpass(wname, tqa, h):
            w = wts[wname]
            dst, bias = (kt, bkt) if wname == "wkT" else (qt, bqt)
            ps = pp_proj.tile([128, QW], F32, tag="proj", name="ps")
            for dc in range(DC):
                nc.tensor.matmul(
                    ps[:],
                    w[:, dc * ES + h * 128 : dc * ES + (h + 1) * 128],
                    xt[:, dc * T + tqa * QW : dc * T + (tqa + 1) * QW],
                    start=(dc == 0), stop=(dc == DC - 1),
                )
            nc.vector.tensor_scalar_add(
                dst[:, h * T + tqa * QW : h * T + (tqa + 1) * QW],
                ps[:], bias[:, h : h + 1],
            )

        def _v_subpass(tqa, tci):
            wv = wts["wvT"]
            ps = pp_proj.tile([128, ES], F32, tag="proj", name="psv")
            tc_i = 4 * tqa + tci
            for dc in range(DC):
                nc.tensor.matmul(
                    ps[:],
                    xt[:, dc * T + tc_i * 128 : dc * T + tc_i * 128 + 128],
                    wv[:, dc * ES : (dc + 1) * ES],
                    start=(dc == 0), stop=(dc == DC - 1),
                )
            nc.vector.tensor_add(
                Vt[:, tc_i * ES : (tc_i + 1) * ES], ps[:], bvb[:]
            )

        def _proj_quarter_units(tqa):
            # each unit is one 16-matmul sub-pass (single PSUM bank, so the
            # 2-buf pool gives a full sub-pass of WAR reuse distance)
            for h in range(H_PER):
                yield lambda t=tqa, h=h: _kq_subpass("wkT", t, h)
            for h in range(H_PER):
                yield lambda t=tqa, h=h: _kq_subpass("wqT", t, h)
            for tci in range(4):
                yield lambda t=tqa, p=tci: _v_subpass(t, p)

        # ---- out-projection for one quarter (emitted as filler groups) ----
        # 2 PSUM banks: eo-pairs, so each (tci, ep) pass accumulates over hc.
        def _op_group(ctxT, t0, tci, ep, hc, psos):
            st = ctxT[:, hc * QW + tci * 128 : hc * QW + tci * 128 + 128]
            for eo in range(2):
                eoa = 2 * ep + eo
                nc.tensor.matmul(
                    psos[eo][:], st,
                    wot[:, hc * D + eoa * ES : hc * D + (eoa + 1) * ES],
                    start=(hc == 0), stop=(hc == H_PER - 1),
                )
            if hc == H_PER - 1:
                ost = os_p.tile([128, 2 * ES], F16, name="ost", tag=f"ost{ep}")
                for eo in range(2):
                    cp = nc.vector.tensor_copy if eo == 0 else nc.scalar.copy
                    cp(ost[:, eo * ES : (eo + 1) * ES], psos[eo][:])
                nc.sync.dma_start(
                    out=out_ap[
                        t0 + tci * 128 : t0 + tci * 128 + 128,
                        2 * ep * ES : 2 * (ep + 1) * ES,
                    ],
                    in_=ost[:],
                )

        def _op_units(ctxT, t0):
            for tci in range(4):
                for ep in range(2):
                    psos = [pp_proj.tile([128, ES], F32, tag="proj",
                                         name=f"pso{eo}")
                            for eo in range(2)]
                    for hc in range(H_PER):
                        yield lambda tci=tci, ep=ep, hc=hc, psos=psos: _op_group(
                            ctxT, t0, tci, ep, hc, psos)

        # ---- attention ----
        _bvb_setup()
        for u in _proj_quarter_units(0):   # quarter-0 K/Q/V, emitted directly
            u()
        filler = list(_proj_quarter_units(1))

        pending_norm = None
        for qi in range(NQ):
            t0 = qi * QW
            nkc = 4 * qi + 4
            ctxT = ctx_p.tile([128, H_PER * QW], F16)  # [hd, h*QW + t_local]
            n_chunks_left = H_PER * nkc
            for h in range(H_PER):
                pctx = pp_ctx.tile([128, QW], F32, tag="ctx", name="pctx")
                pdn = pp_dn.tile([1, QW], F32, tag="dn", name="pdn")
                ets = {}

                def _score(kc, h=h, ets=ets):
                    # scores for key-chunk kc -> exp -> (mask) -> et
                    j = kc - 4 * qi
                    off = max(0, j * 128)
                    w = QW - off
                    psc = pp_sc.tile([128, QW], F32, tag="sc", name="psc")
                    nc.tensor.matmul(
                        psc[:, :w],
                        kt[:, h * T + kc * 128 : h * T + kc * 128 + 128],
                        qt[:, h * T + t0 + off : h * T + t0 + QW],
                        start=True, stop=True,
                    )
                    et = e_p.tile([128, QW], F16)
                    nc.scalar.activation(et[:, :w], psc[:, :w], EXP)
                    if j >= 0:
                        nc.gpsimd.affine_select(
                            out=et[:, :w], in_=et[:, :w],
                            compare_op=mybir.AluOpType.is_ge,
                            fill=0.0, base=0,
                            pattern=[[1, w]], channel_multiplier=-1,
                        )
                    ets[kc] = et

                def _accum(kc, h=h, pctx=pctx, pdn=pdn, ets=ets):
                    # denominator + AV accumulation for key-chunk kc
                    off = max(0, (kc - 4 * qi) * 128)
                    w = QW - off
                    et = ets.pop(kc)
                    nc.tensor.matmul(
                        pdn[:, off:], ones_c[:], et[:, :w],
                        start=(kc == 0), stop=(kc == nkc - 1),
                    )
                    nc.tensor.matmul(
                        pctx[:, off:],
                        Vt[:, kc * ES + h * 128 : kc * ES + (h + 1) * 128],
                        et[:, :w],
                        start=(kc == 0), stop=(kc == nkc - 1),
                    )

                # software pipeline: scores run 2 key-chunks ahead of the
                # dependent accumulation matmuls so the PE never waits on the
                # EXP/mask chain; filler (V / out-proj groups) pads the gaps.
                _score(0)
                if nkc > 1:
                    _score(1)
                for kc in range(nkc):
                    if kc + 2 < nkc:
                        _score(kc + 2)
                    n_pop = (len(filler) + n_chunks_left - 1) // n_chunks_left
                    for _ in range(min(n_pop, len(filler))):
                        filler.pop(0)()
                    n_chunks_left -= 1
                    _accum(kc)
                    if kc == 1 and pending_norm is not None:
                        pending_norm()
                        pending_norm = None

                def _norm(h=h, pctx=pctx, pdn=pdn, ctxT=ctxT):
                    rec = sm_p.tile([1, QW], F32, tag="rec", bufs=1)
                    nc.vector.reciprocal_approx_fast(rec[:], pdn[:])
                    recr = sm_p.tile([1, QW], F16, tag="recr", name="recr", bufs=1)
                    nc.vector.tensor_copy(recr[:], rec[:])
                    pbc = pp_sc.tile([128, QW], F32, tag="sc", name="pbc")
                    nc.tensor.matmul(pbc[:], ones_r[:], recr[:], start=True, stop=True)
                    rb = sm_p.tile([128, QW], F32, tag="rb", name="rb", bufs=1)
                    nc.scalar.copy(rb[:], pbc[:])
                    nc.vector.tensor_mul(
                        ctxT[:, h * QW : (h + 1) * QW], pctx[:], rb[:]
                    )

                pending_norm = _norm
            pending_norm()
            pending_norm = None
            # drain leftover filler (next quarter's projections must be fully
            # emitted before its attention), then queue the next batch:
            # projections of quarter qi+2 plus this quarter's out-projection.
            for u in filler:
                u()
            filler = []
            if qi + 2 < NQ:
                filler += list(_proj_quarter_units(qi + 2))
            filler += list(_op_units(ctxT, t0))

        for u in filler:
            u()

    nc.compile()
    return nc


def _prepare_in_maps(x, wq, bq, wk, bk, wv, bv, wo, bo):
    s = 1.0 / math.sqrt(HD)
    in_maps = []
    for c in range(N_CORES):
        b = c // 4
        g = c % 4
        es = slice(g * ES, (g + 1) * ES)
        in_maps.append(
            {
                "xT": np.ascontiguousarray(x[b].T).astype(np.float16),
                "wqT": np.ascontiguousarray(wq[es, :].T * s).astype(np.float16),
                "wkT": np.ascontiguousarray(wk[es, :].T).astype(np.float16),
                "wvT": np.ascontiguousarray(wv[es, :].T).astype(np.float16),
                "woT": np.ascontiguousarray(wo[:, es].T).astype(np.float16),
                "bq": (bq[es] * s).astype(np.float32).reshape(H_PER, 128, 1),
                "bk": bk[es].astype(np.float32).reshape(H_PER, 128, 1),
                "bv_row": bv[es].astype(np.float32).reshape(1, ES),
                "ones_c": np.ones((128, 1), np.float16),
                "ones_r": np.ones((1, 128), np.float16),
            }
        )
    return in_maps


_CACHED_NC = None


def _get_nc():
    global _CACHED_NC
    if _CACHED_NC is None:
        _CACHED_NC = _build()
    return _CACHED_NC


def kernel(x, wq, bq, wk, bk, wv, bv, wo, bo, _trace=False):
    x, wq, bq, wk, bk, wv, bv, wo, bo = (
        np.asarray(a, np.float32) for a in (x, wq, bq, wk, bk, wv, bv, wo, bo)
    )
    nc = _get_nc()
    in_maps = _prepare_in_maps(x, wq, bq, wk, bk, wv, bv, wo, bo)
    res = run_bass_kernel_spmd(nc, in_maps, list(range(N_CORES)), trace=_trace)
    out = np.zeros((B, T, D), np.float32)
    for b in range(B):
        acc = res.results[4 * b]["partial"].astype(np.float32)
        for g in range(1, 4):
            acc = acc + res.results[4 * b + g]["partial"].astype(np.float32)
        out[b] = acc + bo[None, :]
    if _trace:
        return out, res
    return out


# revision 20
# speedup vs baseline: 1.0193x; 1.0193x over previous
"""Causal self-attention (B=2, T=2048, D=2048, H=16, hd=128) on 8 TRN2 cores.

Sharding: core c = (batch b = c//4, head-group g = c%4).  Each core owns 4
heads (a 512-wide slice of the q/k/v projection outputs and of the out-proj
contraction dim) and one batch.  Each core computes a partial output
(its heads' contribution to x @ wo^T); the host sums the 4 partials per
batch and adds bo.

v2: all matmul operands are fp16 (same 1 cycle/row PE rate as fp32r but
half the DMA bytes and SBUF footprint).  x, and all weights are loaded to
SBUF once.  Q^T/K^T are projected for the FULL sequence per head (4 PSUM
banks = 4x512 columns), V in natural [t, e] layout per 128-row chunk.
Scores are computed transposed ([k, q]) so softmax needs no transposes of
P; the denominator comes from a ones-vector matmul.  exp() runs without
max-subtraction (score range ~N(0, 0.33)).

Causal trim: for the diagonal (same-quarter) key chunks the moving query
range is restricted to the valid suffix, so scores/exp/denominator/AV all
skip the upper triangle at 128-granularity.

PE bubbles in the attention dependency chain (scores -> EXP -> AV) are
plugged with filler matmul groups: the V projection during quarter 0's
attention, the previous quarter's out-projection during later quarters.
"""
import math
from contextlib import ExitStack

import numpy as np

import concourse.bass as bass
import concourse.tile as tile
from concourse import bacc, mybir
from concourse.bass_utils import run_bass_kernel_spmd

D = 2048
T = 2048
B = 2
HD = 128          # head dim
H_PER = 4         # heads per core
ES = 512          # e-slice width per core (H_PER * HD)
NQ = 4            # time quarters
QW = T // NQ      # quarter width (512)
DC = D // 128     # d-chunks (16)
TC = T // 128     # t-chunks (16)
N_CORES = 8

F32 = mybir.dt.float32
F16 = mybir.dt.float16
EXP = mybir.ActivationFunctionType.Exp


def _build():
    nc = bacc.Bacc("TRN2", target_bir_lowering=False, debug=False)
    dram = {}
    for name, shape, dt in [
        ("xT", [D, T], F16),
        ("wqT", [D, ES], F16),
        ("wkT", [D, ES], F16),
        ("wvT", [D, ES], F16),
        ("woT", [ES, D], F16),
        ("bq", [H_PER, 128], F32),
        ("bk", [H_PER, 128], F32),
        ("bv_row", [1, ES], F32),
        ("ones_c", [128, 1], F16),
        ("ones_r", [1, 128], F16),
    ]:
        dram[name] = nc.dram_tensor(name, shape, dt, kind="ExternalInput").ap()
    out_ap = nc.dram_tensor("partial", [T, D], F16, kind="ExternalOutput").ap()

    with tile.TileContext(nc) as tc, ExitStack() as ctx:
        const_p = ctx.enter_context(tc.tile_pool(name="const", bufs=1))
        big_p = ctx.enter_context(tc.tile_pool(name="big", bufs=1))
        e_p = ctx.enter_context(tc.tile_pool(name="expT", bufs=4))
        ctx_p = ctx.enter_context(tc.tile_pool(name="ctxT", bufs=2))
        os_p = ctx.enter_context(tc.tile_pool(name="ostage", bufs=2))
        sm_p = ctx.enter_context(tc.tile_pool(name="small", bufs=2))
        pp_proj = ctx.enter_context(tc.tile_pool(name="pproj", bufs=2, space="PSUM"))
        pp_sc = ctx.enter_context(tc.tile_pool(name="psc", bufs=3, space="PSUM"))
        pp_ctx = ctx.enter_context(tc.tile_pool(name="pctx", bufs=2, space="PSUM"))
        pp_dn = ctx.enter_context(tc.tile_pool(name="pdn", bufs=1, space="PSUM"))

        # ---- constants ----
        bqt = const_p.tile([128, H_PER], F32, tag="bq")
        bkt = const_p.tile([128, H_PER], F32, tag="bk")
        nc.sync.dma_start(out=bqt[:], in_=dram["bq"].rearrange("h p -> p h"))
        nc.sync.dma_start(out=bkt[:], in_=dram["bk"].rearrange("h p -> p h"))
        ones_c = const_p.tile([128, 1], F16, tag="ones_c")
        nc.sync.dma_start(out=ones_c[:], in_=dram["ones_c"][:])
        ones_r = const_p.tile([1, 128], F16, tag="ones_r")
        nc.sync.dma_start(out=ones_r[:], in_=dram["ones_r"][:])
        bv_row = const_p.tile([1, ES], F32, tag="bv_row")
        nc.sync.dma_start(out=bv_row[:], in_=dram["bv_row"][:])
        bvb = const_p.tile([128, ES], F32, tag="bvb")

        def _bvb_setup():
            # deferred so this chain never blocks the head of the PE queue
            bvr_r = const_p.tile([1, ES], F16, tag="bvr_r")
            nc.vector.tensor_copy(bvr_r[:], bv_row[:])
            pbv = pp_sc.tile([128, ES], F32, tag="sc", name="pbv")
            nc.tensor.matmul(pbv[:], ones_r[:], bvr_r[:], start=True, stop=True)
            nc.scalar.copy(bvb[:], pbv[:])

        # ---- bulk loads: weights (sync queue), x^T (gpsimd queue, streamed
        # quarter-major so quarter-0 projections start early) ----
        # xt[:, dc*T + t] = xT[dc*128 + p, t]
        xt = big_p.tile([128, DC * T], F16, tag="xt")
        for dc in range(DC):   # quarter-0 pieces: fine-grained, land first
            nc.gpsimd.dma_start(
                out=xt[:, dc * T : dc * T + QW],
                in_=dram["xT"][dc * 128 : (dc + 1) * 128, 0:QW],
            )
        wts = {}
        for name in ("wkT", "wqT", "wvT"):
            w = big_p.tile([128, DC * ES], F16, tag=name, name=name + "_t")
            nc.sync.dma_start(
                out=w[:].rearrange("p (dc e) -> p dc e", e=ES),
                in_=dram[name].rearrange("(dc p) e -> p dc e", p=128),
            )
            wts[name] = w
        # wot[:, hc*D + eo] = woT[hc*128 + p, eo]
        wot = big_p.tile([128, H_PER * D], F16, tag="wot")
        nc.sync.dma_start(
            out=wot[:].rearrange("p (hc d) -> p hc d", d=D),
            in_=dram["woT"].rearrange("(hc p) d -> p hc d", p=128),
        )
        # x quarters 1-3: one big strided DMA on the sync queue
        nc.sync.dma_start(
            out=xt[:].rearrange("p (dc t) -> p dc t", t=T)[:, :, QW:T],
            in_=dram["xT"].rearrange("(dc p) t -> p dc t", p=128)[:, :, QW:T],
        )

        # ---- projections, one quarter of t at a time (2-bank sub-passes) ----
        # qt/kt[:, h*T + t] : partition = head-dim
        # Vt[:, tc*ES + e]  : partition = t within chunk tc
        qt = big_p.tile([128, H_PER * T], F16, tag="qt")
        kt = big_p.tile([128, H_PER * T], F16, tag="kt")
        Vt = big_p.tile([128, TC * ES], F16, tag="Vt")

        def _kq_subpass(wname, tqa, h):
            w = wts[wname]
            dst, bias = (kt, bkt) if wname == "wkT" else (qt, bqt)
            ps = pp_proj.tile([128, QW], F32, tag="proj", name="ps")
            for dc in range(DC):
                nc.tensor.matmul(
                    ps[:],
                    w[:, dc * ES + h * 128 : dc * ES + (h + 1) * 128],
                    xt[:, dc * T + tqa * QW : dc * T + (tqa + 1) * QW],
                    start=(dc == 0), stop=(dc == DC - 1),
                )
            nc.vector.tensor_scalar_add(
                dst[:, h * T + tqa * QW : h * T + (tqa + 1) * QW],
                ps[:], bias[:, h : h + 1],
            )

        def _v_subpass(tqa, tci):
            wv = wts["wvT"]
            ps = pp_proj.tile([128, ES], F32, tag="proj", name="psv")
            tc_i = 4 * tqa + tci
            for dc in range(DC):
                nc.tensor.matmul(
                    ps[:],
                    xt[:, dc * T + tc_i * 128 : dc * T + tc_i * 128 + 128],
                    wv[:, dc * ES : (dc + 1) * ES],
                    start=(dc == 0), stop=(dc == DC - 1),
                )
            nc.vector.tensor_add(
                Vt[:, tc_i * ES : (tc_i + 1) * ES], ps[:], bvb[:]
            )

        def _proj_quarter_units(tqa):
            # each unit is one 16-matmul sub-pass (single PSUM bank, so the
            # 2-buf pool gives a full sub-pass of WAR reuse distance)
            for h in range(H_PER):
                yield lambda t=tqa, h=h: _kq_subpass("wkT", t, h)
            for h in range(H_PER):
                yield lambda t=tqa, h=h: _kq_subpass("wqT", t, h)
            for tci in range(4):
                yield lambda t=tqa, p=tci: _v_subpass(t, p)

        # ---- out-projection for one quarter (emitted as filler groups) ----
        # 2 PSUM banks: eo-pairs, so each (tci, ep) pass accumulates over hc.
        def _op_group(ctxT, t0, tci, ep, hc, psos):
            st = ctxT[:, hc * QW + tci * 128 : hc * QW + tci * 128 + 128]
            for eo in range(2):
                eoa = 2 * ep + eo
                nc.tensor.matmul(
                    psos[eo][:], st,
                    wot[:, hc * D + eoa * ES : hc * D + (eoa + 1) * ES],
                    start=(hc == 0), stop=(hc == H_PER - 1),
                )
            if hc == H_PER - 1:
                ost = os_p.tile([128, 2 * ES], F16, name="ost", tag=f"ost{ep}")
                for eo in range(2):
                    cp = nc.vector.tensor_copy if eo == 0 else nc.scalar.copy
                    cp(ost[:, eo * ES : (eo + 1) * ES], psos[eo][:])
                nc.sync.dma_start(
                    out=out_ap[
                        t0 + tci * 128 : t0 + tci * 128 + 128,
                        2 * ep * ES : 2 * (ep + 1) * ES,
                    ],
                    in_=ost[:],
                )

        def _op_units(ctxT, t0):
            for tci in range(4):
                for ep in range(2):
                    psos = [pp_proj.tile([128, ES], F32, tag="proj",
                                         name=f"pso{eo}")
                            for eo in range(2)]
                    for hc in range(H_PER):
                        yield lambda tci=tci, ep=ep, hc=hc, psos=psos: _op_group(
                            ctxT, t0, tci, ep, hc, psos)

        # ---- attention ----
        _bvb_setup()
        for u in _proj_quarter_units(0):   # quarter-0 K/Q/V, emitted directly
            u()
        filler = list(_proj_quarter_units(1))

        pending_norm = None
        for qi in range(NQ):
            t0 = qi * QW
            nkc = 4 * qi + 4
            ctxT = ctx_p.tile([128, H_PER * QW], F16)  # [hd, h*QW + t_local]
            n_chunks_left = H_PER * nkc
            for h in range(H_PER):
                pctx = pp_ctx.tile([128, QW], F32, tag="ctx", name="pctx")
                pdn = pp_dn.tile([1, QW], F32, tag="dn", name="pdn")
                ets = {}

                def _score(kc, h=h, ets=ets):
                    # scores for key-chunk kc -> exp -> (mask) -> et
                    j = kc - 4 * qi
                    off = max(0, j * 128)
                    w = QW - off
                    psc = pp_sc.tile([128, QW], F32, tag="sc", name="psc")
                    nc.tensor.matmul(
                        psc[:, :w],
                        kt[:, h * T + kc * 128 : h * T + kc * 128 + 128],
                        qt[:, h * T + t0 + off : h * T + t0 + QW],
                        start=True, stop=True,
                    )
                    et = e_p.tile([128, QW], F16)
                    nc.scalar.activation(et[:, :w], psc[:, :w], EXP)
                    if j >= 0:
                        nc.gpsimd.affine_select(
                            out=et[:, :w], in_=et[:, :w],
                            compare_op=mybir.AluOpType.is_ge,
                            fill=0.0, base=0,
                            pattern=[[1, w]], channel_multiplier=-1,
                        )
                    ets[kc] = et

                def _accum(kc, h=h, pctx=pctx, pdn=pdn, ets=ets):
                    # denominator + AV accumulation for key-chunk kc
                    off = max(0, (kc - 4 * qi) * 128)
                    w = QW - off
                    et = ets.pop(kc)
                    nc.tensor.matmul(
                        pdn[:, off:], ones_c[:], et[:, :w],
                        start=(kc == 0), stop=(kc == nkc - 1),
                    )
                    nc.tensor.matmul(
                        pctx[:, off:],
                        Vt[:, kc * ES + h * 128 : kc * ES + (h + 1) * 128],
                        et[:, :w],
                        start=(kc == 0), stop=(kc == nkc - 1),
                    )

                # software pipeline: scores run 2 key-chunks ahead of the
                # dependent accumulation matmuls so the PE never waits on the
                # EXP/mask chain; filler (V / out-proj groups) pads the gaps.
                _score(0)
                if nkc > 1:
                    _score(1)
                for kc in range(nkc):
                    if kc + 2 < nkc:
                        _score(kc + 2)
                    n_pop = (len(filler) + n_chunks_left - 1) // n_chunks_left
                    for _ in range(min(n_pop, len(filler))):
                        filler.pop(0)()
                    n_chunks_left -= 1
                    _accum(kc)
                    if kc == 1 and pending_norm is not None:
                        pending_norm()
                        pending_norm = None

                def _norm(h=h, pctx=pctx, pdn=pdn, ctxT=ctxT):
                    rec = sm_p.tile([1, QW], F32, tag="rec", bufs=1)
                    nc.vector.reciprocal_approx_fast(rec[:], pdn[:])
                    recr = sm_p.tile([1, QW], F16, tag="recr", name="recr", bufs=1)
                    nc.vector.tensor_copy(recr[:], rec[:])
                    pbc = pp_sc.tile([128, QW], F32, tag="sc", name="pbc")
                    nc.tensor.matmul(pbc[:], ones_r[:], recr[:], start=True, stop=True)
                    rb = sm_p.tile([128, QW], F32, tag="rb", name="rb", bufs=1)
                    nc.scalar.copy(rb[:], pbc[:])
                    nc.vector.tensor_mul(
                        ctxT[:, h * QW : (h + 1) * QW], pctx[:], rb[:]
                    )

                pending_norm = _norm
            pending_norm()
            pending_norm = None
            # drain leftover filler (next quarter's projections must be fully
            # emitted before its attention), then queue the next batch:
            # projections of quarter qi+2 plus this quarter's out-projection.
            for u in filler:
                u()
            filler = []
            if qi + 2 < NQ:
                filler += list(_proj_quarter_units(qi + 2))
            filler += list(_op_units(ctxT, t0))

        for u in filler:
            u()

    nc.compile()
    return nc


def _prepare_in_maps(x, wq, bq, wk, bk, wv, bv, wo, bo):
    s = 1.0 / math.sqrt(HD)
    in_maps = []
    for c in range(N_CORES):
        b = c // 4
        g = c % 4
        es = slice(g * ES, (g + 1) * ES)
        in_maps.append(
            {
                "xT": np.ascontiguousarray(x[b].T).astype(np.float16),
                "wqT": np.ascontiguousarray(wq[es, :].T * s).astype(np.float16),
                "wkT": np.ascontiguousarray(wk[es, :].T).astype(np.float16),
                "wvT": np.ascontiguousarray(wv[es, :].T).astype(np.float16),
                "woT": np.ascontiguousarray(wo[:, es].T).astype(np.float16),
                "bq": (bq[es] * s).astype(np.float32).reshape(H_PER, 128),
                "bk": bk[es].astype(np.float32).reshape(H_PER, 128),
                "bv_row": bv[es].astype(np.float32).reshape(1, ES),
                "ones_c": np.ones((128, 1), np.float16),
                "ones_r": np.ones((1, 128), np.float16),
            }
        )
    return in_maps


_CACHED_NC = None


def _get_nc():
    global _CACHED_NC
    if _CACHED_NC is None:
        _CACHED_NC = _build()
    return _CACHED_NC


def kernel(x, wq, bq, wk, bk, wv, bv, wo, bo, _trace=False):
    x, wq, bq, wk, bk, wv, bv, wo, bo = (
        np.asarray(a, np.float32) for a in (x, wq, bq, wk, bk, wv, bv, wo, bo)
    )
    nc = _get_nc()
    in_maps = _prepare_in_maps(x, wq, bq, wk, bk, wv, bv, wo, bo)
    res = run_bass_kernel_spmd(nc, in_maps, list(range(N_CORES)), trace=_trace)
    out = np.zeros((B, T, D), np.float32)
    for b in range(B):
        acc = res.results[4 * b]["partial"].astype(np.float32)
        for g in range(1, 4):
            acc = acc + res.results[4 * b + g]["partial"].astype(np.float32)
        out[b] = acc + bo[None, :]
    if _trace:
        return out, res
    return out


# revision 23
# speedup vs baseline: 1.0274x; 1.0080x over previous
"""Causal self-attention (B=2, T=2048, D=2048, H=16, hd=128) on 8 TRN2 cores.

Sharding: core c = (batch b = c//4, head-group g = c%4).  Each core owns 4
heads (a 512-wide slice of the q/k/v projection outputs and of the out-proj
contraction dim) and one batch.  Each core computes a partial output
(its heads' contribution to x @ wo^T); the host sums the 4 partials per
batch and adds bo.

v2: all matmul operands are fp16 (same 1 cycle/row PE rate as fp32r but
half the DMA bytes and SBUF footprint).  x, and all weights are loaded to
SBUF once.  Q^T/K^T are projected for the FULL sequence per head (4 PSUM
banks = 4x512 columns), V in natural [t, e] layout per 128-row chunk.
Scores are computed transposed ([k, q]) so softmax needs no transposes of
P; the denominator comes from a ones-vector matmul.  exp() runs without
max-subtraction (score range ~N(0, 0.33)).

Causal trim: for the diagonal (same-quarter) key chunks the moving query
range is restricted to the valid suffix, so scores/exp/denominator/AV all
skip the upper triangle at 128-granularity.

PE bubbles in the attention dependency chain (scores -> EXP -> AV) are
plugged with filler matmul groups: the V projection during quarter 0's
attention, the previous quarter's out-projection during later quarters.
"""
import math
from contextlib import ExitStack

import numpy as np

import concourse.bass as bass
import concourse.tile as tile
from concourse import bacc, mybir
from concourse.bass_utils import run_bass_kernel_spmd

D = 2048
T = 2048
B = 2
HD = 128          # head dim
H_PER = 4         # heads per core
ES = 512          # e-slice width per core (H_PER * HD)
NQ = 4            # time quarters
QW = T // NQ      # quarter width (512)
DC = D // 128     # d-chunks (16)
TC = T // 128     # t-chunks (16)
N_CORES = 8

F32 = mybir.dt.float32
F16 = mybir.dt.float16
EXP = mybir.ActivationFunctionType.Exp


def _build():
    nc = bacc.Bacc("TRN2", target_bir_lowering=False, debug=False)
    dram = {}
    for name, shape, dt in [
        ("xT", [D, T], F16),
        ("wqT", [D, ES], F16),
        ("wkT", [D, ES], F16),
        ("wvT", [D, ES], F16),
        ("woT", [ES, D], F16),
        ("bq", [H_PER, 128], F32),
        ("bk", [H_PER, 128], F32),
        ("bv_row", [1, ES], F32),
        ("ones_c", [128, 1], F16),
        ("ones_r", [1, 128], F16),
    ]:
        dram[name] = nc.dram_tensor(name, shape, dt, kind="ExternalInput").ap()
    out_ap = nc.dram_tensor("partial", [T, D], F16, kind="ExternalOutput").ap()

    with tile.TileContext(nc) as tc, ExitStack() as ctx:
        const_p = ctx.enter_context(tc.tile_pool(name="const", bufs=1))
        big_p = ctx.enter_context(tc.tile_pool(name="big", bufs=1))
        e_p = ctx.enter_context(tc.tile_pool(name="expT", bufs=4))
        ctx_p = ctx.enter_context(tc.tile_pool(name="ctxT", bufs=2))
        os_p = ctx.enter_context(tc.tile_pool(name="ostage", bufs=2))
        sm_p = ctx.enter_context(tc.tile_pool(name="small", bufs=2))
        pp_proj = ctx.enter_context(tc.tile_pool(name="pproj", bufs=2, space="PSUM"))
        pp_sc = ctx.enter_context(tc.tile_pool(name="psc", bufs=3, space="PSUM"))
        pp_ctx = ctx.enter_context(tc.tile_pool(name="pctx", bufs=2, space="PSUM"))
        pp_dn = ctx.enter_context(tc.tile_pool(name="pdn", bufs=1, space="PSUM"))

        # ---- constants ----
        bqt = const_p.tile([128, H_PER], F32, tag="bq")
        bkt = const_p.tile([128, H_PER], F32, tag="bk")
        nc.sync.dma_start(out=bqt[:], in_=dram["bq"].rearrange("h p -> p h"))
        nc.sync.dma_start(out=bkt[:], in_=dram["bk"].rearrange("h p -> p h"))
        ones_c = const_p.tile([128, 1], F16, tag="ones_c")
        nc.sync.dma_start(out=ones_c[:], in_=dram["ones_c"][:])
        ones_r = const_p.tile([1, 128], F16, tag="ones_r")
        nc.sync.dma_start(out=ones_r[:], in_=dram["ones_r"][:])
        bv_row = const_p.tile([1, ES], F32, tag="bv_row")
        nc.sync.dma_start(out=bv_row[:], in_=dram["bv_row"][:])
        bvb = const_p.tile([128, ES], F32, tag="bvb")

        def _bvb_setup():
            # deferred so this chain never blocks the head of the PE queue
            bvr_r = const_p.tile([1, ES], F16, tag="bvr_r")
            nc.vector.tensor_copy(bvr_r[:], bv_row[:])
            pbv = pp_sc.tile([128, ES], F32, tag="sc", name="pbv")
            nc.tensor.matmul(pbv[:], ones_r[:], bvr_r[:], start=True, stop=True)
            nc.scalar.copy(bvb[:], pbv[:])

        # ---- bulk loads: weights (sync queue), x^T (gpsimd queue, streamed
        # quarter-major so quarter-0 projections start early) ----
        # xt[:, dc*T + t] = xT[dc*128 + p, t]
        xt = big_p.tile([128, DC * T], F16, tag="xt")
        for dc in range(DC):   # quarter-0 pieces: fine-grained, land first
            nc.gpsimd.dma_start(
                out=xt[:, dc * T : dc * T + QW],
                in_=dram["xT"][dc * 128 : (dc + 1) * 128, 0:QW],
            )
        wts = {}
        for name, nsplit in (("wkT", 4), ("wqT", 2), ("wvT", 2)):
            w = big_p.tile([128, DC * ES], F16, tag=name, name=name + "_t")
            step = DC // nsplit
            for s in range(nsplit):
                nc.sync.dma_start(
                    out=w[:].rearrange("p (dc e) -> p dc e", e=ES)
                        [:, s * step : (s + 1) * step],
                    in_=dram[name].rearrange("(dc p) e -> p dc e", p=128)
                        [:, s * step : (s + 1) * step],
                )
            wts[name] = w
        # wot[:, hc*D + eo] = woT[hc*128 + p, eo]
        wot = big_p.tile([128, H_PER * D], F16, tag="wot")
        nc.sync.dma_start(
            out=wot[:].rearrange("p (hc d) -> p hc d", d=D),
            in_=dram["woT"].rearrange("(hc p) d -> p hc d", p=128),
        )
        # x quarters 1-3: one big strided DMA on the sync queue
        nc.sync.dma_start(
            out=xt[:].rearrange("p (dc t) -> p dc t", t=T)[:, :, QW:T],
            in_=dram["xT"].rearrange("(dc p) t -> p dc t", p=128)[:, :, QW:T],
        )

        # ---- projections, one quarter of t at a time (2-bank sub-passes) ----
        # qt/kt[:, h*T + t] : partition = head-dim
        # Vt[:, tc*ES + e]  : partition = t within chunk tc
        qt = big_p.tile([128, H_PER * T], F16, tag="qt")
        kt = big_p.tile([128, H_PER * T], F16, tag="kt")
        Vt = big_p.tile([128, TC * ES], F16, tag="Vt")

        def _kq_subpass(wname, tqa, h):
            w = wts[wname]
            dst, bias = (kt, bkt) if wname == "wkT" else (qt, bqt)
            ps = pp_proj.tile([128, QW], F32, tag="proj", name="ps")
            for dc in range(DC):
                nc.tensor.matmul(
                    ps[:],
                    w[:, dc * ES + h * 128 : dc * ES + (h + 1) * 128],
                    xt[:, dc * T + tqa * QW : dc * T + (tqa + 1) * QW],
                    start=(dc == 0), stop=(dc == DC - 1),
                )
            nc.vector.tensor_scalar_add(
                dst[:, h * T + tqa * QW : h * T + (tqa + 1) * QW],
                ps[:], bias[:, h : h + 1],
            )

        def _v_subpass(tqa, tci):
            wv = wts["wvT"]
            ps = pp_proj.tile([128, ES], F32, tag="proj", name="psv")
            tc_i = 4 * tqa + tci
            for dc in range(DC):
                nc.tensor.matmul(
                    ps[:],
                    xt[:, dc * T + tc_i * 128 : dc * T + tc_i * 128 + 128],
                    wv[:, dc * ES : (dc + 1) * ES],
                    start=(dc == 0), stop=(dc == DC - 1),
                )
            nc.vector.tensor_add(
                Vt[:, tc_i * ES : (tc_i + 1) * ES], ps[:], bvb[:]
            )

        def _proj_quarter_units(tqa):
            # each unit is one 16-matmul sub-pass (single PSUM bank, so the
            # 2-buf pool gives a full sub-pass of WAR reuse distance)
            for h in range(H_PER):
                yield lambda t=tqa, h=h: _kq_subpass("wkT", t, h)
            for h in range(H_PER):
                yield lambda t=tqa, h=h: _kq_subpass("wqT", t, h)
            for tci in range(4):
                yield lambda t=tqa, p=tci: _v_subpass(t, p)

        # ---- out-projection for one quarter (emitted as filler groups) ----
        # 2 PSUM banks: eo-pairs, so each (tci, ep) pass accumulates over hc.
        def _op_group(ctxT, t0, tci, ep, hc, psos):
            st = ctxT[:, hc * QW + tci * 128 : hc * QW + tci * 128 + 128]
            for eo in range(2):
                eoa = 2 * ep + eo
                nc.tensor.matmul(
                    psos[eo][:], st,
                    wot[:, hc * D + eoa * ES : hc * D + (eoa + 1) * ES],
                    start=(hc == 0), stop=(hc == H_PER - 1),
                )
            if hc == H_PER - 1:
                ost = os_p.tile([128, 2 * ES], F16, name="ost", tag=f"ost{ep}")
                for eo in range(2):
                    cp = nc.vector.tensor_copy if eo == 0 else nc.scalar.copy
                    cp(ost[:, eo * ES : (eo + 1) * ES], psos[eo][:])
                dma = nc.sync.dma_start if (tci + ep) % 2 == 0 else nc.gpsimd.dma_start
                dma(
                    out=out_ap[
                        t0 + tci * 128 : t0 + tci * 128 + 128,
                        2 * ep * ES : 2 * (ep + 1) * ES,
                    ],
                    in_=ost[:],
                )

        def _op_units(ctxT, t0):
            for tci in range(4):
                for ep in range(2):
                    psos = [pp_proj.tile([128, ES], F32, tag="proj",
                                         name=f"pso{eo}")
                            for eo in range(2)]
                    for hc in range(H_PER):
                        yield lambda tci=tci, ep=ep, hc=hc, psos=psos: _op_group(
                            ctxT, t0, tci, ep, hc, psos)

        # ---- attention ----
        q0_units = list(_proj_quarter_units(0))
        q0_units[0]()                      # first K sub-pass leads the PE queue
        _bvb_setup()                       # only needed by the V sub-passes
        for u in q0_units[1:]:             # rest of quarter-0 K/Q/V
            u()
        filler = list(_proj_quarter_units(1))

        pending_norm = None
        for qi in range(NQ):
            t0 = qi * QW
            nkc = 4 * qi + 4
            ctxT = ctx_p.tile([128, H_PER * QW], F16)  # [hd, h*QW + t_local]
            n_chunks_left = H_PER * nkc
            for h in range(H_PER):
                pctx = pp_ctx.tile([128, QW], F32, tag="ctx", name="pctx")
                pdn = pp_dn.tile([1, QW], F32, tag="dn", name="pdn")
                ets = {}

                def _score(kc, h=h, ets=ets):
                    # scores for key-chunk kc -> exp -> (mask) -> et
                    j = kc - 4 * qi
                    off = max(0, j * 128)
                    w = QW - off
                    psc = pp_sc.tile([128, QW], F32, tag="sc", name="psc")
                    nc.tensor.matmul(
                        psc[:, :w],
                        kt[:, h * T + kc * 128 : h * T + kc * 128 + 128],
                        qt[:, h * T + t0 + off : h * T + t0 + QW],
                        start=True, stop=True,
                    )
                    et = e_p.tile([128, QW], F16)
                    nc.scalar.activation(et[:, :w], psc[:, :w], EXP)
                    if j >= 0:
                        nc.gpsimd.affine_select(
                            out=et[:, :w], in_=et[:, :w],
                            compare_op=mybir.AluOpType.is_ge,
                            fill=0.0, base=0,
                            pattern=[[1, w]], channel_multiplier=-1,
                        )
                    ets[kc] = et

                def _accum(kc, h=h, pctx=pctx, pdn=pdn, ets=ets):
                    # denominator + AV accumulation for key-chunk kc
                    off = max(0, (kc - 4 * qi) * 128)
                    w = QW - off
                    et = ets.pop(kc)
                    nc.tensor.matmul(
                        pdn[:, off:], ones_c[:], et[:, :w],
                        start=(kc == 0), stop=(kc == nkc - 1),
                    )
                    nc.tensor.matmul(
                        pctx[:, off:],
                        Vt[:, kc * ES + h * 128 : kc * ES + (h + 1) * 128],
                        et[:, :w],
                        start=(kc == 0), stop=(kc == nkc - 1),
                    )

                # software pipeline: scores run 2 key-chunks ahead of the
                # dependent accumulation matmuls so the PE never waits on the
                # EXP/mask chain; filler (V / out-proj groups) pads the gaps.
                _score(0)
                if nkc > 1:
                    _score(1)
                for kc in range(nkc):
                    if kc + 2 < nkc:
                        _score(kc + 2)
                    n_pop = (len(filler) + n_chunks_left - 1) // n_chunks_left
                    for _ in range(min(n_pop, len(filler))):
                        filler.pop(0)()
                    n_chunks_left -= 1
                    _accum(kc)
                    if kc == 1 and pending_norm is not None:
                        pending_norm()
                        pending_norm = None

                def _norm(h=h, pctx=pctx, pdn=pdn, ctxT=ctxT):
                    rec = sm_p.tile([1, QW], F32, tag="rec", bufs=1)
                    nc.vector.reciprocal_approx_fast(rec[:], pdn[:])
                    recr = sm_p.tile([1, QW], F16, tag="recr", name="recr", bufs=1)
                    nc.vector.tensor_copy(recr[:], rec[:])
                    pbc = pp_sc.tile([128, QW], F32, tag="sc", name="pbc")
                    nc.tensor.matmul(pbc[:], ones_r[:], recr[:], start=True, stop=True)
                    rb = sm_p.tile([128, QW], F32, tag="rb", name="rb", bufs=1)
                    nc.scalar.copy(rb[:], pbc[:])
                    nc.vector.tensor_mul(
                        ctxT[:, h * QW : (h + 1) * QW], pctx[:], rb[:]
                    )

                pending_norm = _norm
            pending_norm()
            pending_norm = None
            # drain leftover filler (next quarter's projections must be fully
            # emitted before its attention), then queue the next batch:
            # projections of quarter qi+2 plus this quarter's out-projection.
            for u in filler:
                u()
            filler = []
            if qi + 2 < NQ:
                filler += list(_proj_quarter_units(qi + 2))
            filler += list(_op_units(ctxT, t0))

        for u in filler:
            u()

    nc.compile()
    return nc


def _prepare_in_maps(x, wq, bq, wk, bk, wv, bv, wo, bo):
    s = 1.0 / math.sqrt(HD)
    in_maps = []
    for c in range(N_CORES):
        b = c // 4
        g = c % 4
        es = slice(g * ES, (g + 1) * ES)
        in_maps.append(
            {
                "xT": np.ascontiguousarray(x[b].T).astype(np.float16),
                "wqT": np.ascontiguousarray(wq[es, :].T * s).astype(np.float16),
                "wkT": np.ascontiguousarray(wk[es, :].T).astype(np.float16),
                "wvT": np.ascontiguousarray(wv[es, :].T).astype(np.float16),
                "woT": np.ascontiguousarray(wo[:, es].T).astype(np.float16),
                "bq": (bq[es] * s).astype(np.float32).reshape(H_PER, 128),
                "bk": bk[es].astype(np.float32).reshape(H_PER, 128),
                "bv_row": bv[es].astype(np.float32).reshape(1, ES),
                "ones_c": np.ones((128, 1), np.float16),
                "ones_r": np.ones((1, 128), np.float16),
            }
        )
    return in_maps


_CACHED_NC = None


def _get_nc():
    global _CACHED_NC
    if _CACHED_NC is None:
        _CACHED_NC = _build()
    return _CACHED_NC


def kernel(x, wq, bq, wk, bk, wv, bv, wo, bo, _trace=False):
    x, wq, bq, wk, bk, wv, bv, wo, bo = (
        np.asarray(a, np.float32) for a in (x, wq, bq, wk, bk, wv, bv, wo, bo)
    )
    nc = _get_nc()
    in_maps = _prepare_in_maps(x, wq, bq, wk, bk, wv, bv, wo, bo)
    res = run_bass_kernel_spmd(nc, in_maps, list(range(N_CORES)), trace=_trace)
    out = np.zeros((B, T, D), np.float32)
    for b in range(B):
        acc = res.results[4 * b]["partial"].astype(np.float32)
        for g in range(1, 4):
            acc = acc + res.results[4 * b + g]["partial"].astype(np.float32)
        out[b] = acc + bo[None, :]
    if _trace:
        return out, res
    return out


# revision 28
# speedup vs baseline: 1.1493x; 1.1186x over previous
"""Causal self-attention (B=2, T=2048, D=2048, H=16, hd=128) on 8 TRN2 cores.

Sharding: core c = (batch b = c//4, head-group g = c%4).  Each core owns 4
heads (a 512-wide slice of the q/k/v projection outputs and of the out-proj
contraction dim) and one batch.  Each core computes a partial output
(its heads' contribution to x @ wo^T); the host sums the 4 partials per
batch and adds bo.

v2: all matmul operands are fp16 (same 1 cycle/row PE rate as fp32r but
half the DMA bytes and SBUF footprint).  x, and all weights are loaded to
SBUF once.  Q^T/K^T are projected for the FULL sequence per head (4 PSUM
banks = 4x512 columns), V in natural [t, e] layout per 128-row chunk.
Scores are computed transposed ([k, q]) so softmax needs no transposes of
P; the denominator comes from a ones-vector matmul.  exp() runs without
max-subtraction (score range ~N(0, 0.33)).

Causal trim: for the diagonal (same-quarter) key chunks the moving query
range is restricted to the valid suffix, so scores/exp/denominator/AV all
skip the upper triangle at 128-granularity.

PE bubbles in the attention dependency chain (scores -> EXP -> AV) are
plugged with filler matmul groups: the V projection during quarter 0's
attention, the previous quarter's out-projection during later quarters.
"""
import math
from contextlib import ExitStack

import numpy as np

import concourse.bass as bass
import concourse.tile as tile
from concourse import bacc, mybir
from concourse.bass_utils import run_bass_kernel_spmd

D = 2048
T = 2048
B = 2
HD = 128          # head dim
H_PER = 4         # heads per core
ES = 512          # e-slice width per core (H_PER * HD)
NQ = 4            # time quarters
QW = T // NQ      # quarter width (512)
DC = D // 128     # d-chunks (16)
TC = T // 128     # t-chunks (16)
N_CORES = 8

F32 = mybir.dt.float32
F16 = mybir.dt.float16
EXP = mybir.ActivationFunctionType.Exp


def _build():
    nc = bacc.Bacc("TRN2", target_bir_lowering=False, debug=False)
    dram = {}
    for name, shape, dt in [
        ("xT", [D, T], F16),
        ("wqT", [D, ES], F16),
        ("wkT", [D, ES], F16),
        ("wvT", [D, ES], F16),
        ("woT", [ES, D], F16),
        ("bq", [H_PER, 128], F32),
        ("bk", [H_PER, 128], F32),
        ("bv_row", [1, ES], F32),
        ("ones_r", [1, 128], F16),
    ]:
        dram[name] = nc.dram_tensor(name, shape, dt, kind="ExternalInput").ap()
    out_ap = nc.dram_tensor("partial", [T, D], F16, kind="ExternalOutput").ap()

    with tile.TileContext(nc) as tc, ExitStack() as ctx:
        const_p = ctx.enter_context(tc.tile_pool(name="const", bufs=1))
        big_p = ctx.enter_context(tc.tile_pool(name="big", bufs=1))
        e_p = ctx.enter_context(tc.tile_pool(name="expT", bufs=4))
        ctx_p = ctx.enter_context(tc.tile_pool(name="ctxT", bufs=2))
        os_p = ctx.enter_context(tc.tile_pool(name="ostage", bufs=2))
        sm_p = ctx.enter_context(tc.tile_pool(name="small", bufs=2))
        pp_proj = ctx.enter_context(tc.tile_pool(name="pproj", bufs=2, space="PSUM"))
        pp_sc = ctx.enter_context(tc.tile_pool(name="psc", bufs=3, space="PSUM"))
        pp_ctx = ctx.enter_context(tc.tile_pool(name="pctx", bufs=2, space="PSUM"))
        pp_dn = ctx.enter_context(tc.tile_pool(name="pdn", bufs=1, space="PSUM"))

        # ---- constants ----
        bqt = const_p.tile([128, H_PER], F32, tag="bq")
        bkt = const_p.tile([128, H_PER], F32, tag="bk")
        nc.sync.dma_start(out=bqt[:], in_=dram["bq"].rearrange("h p -> p h"))
        nc.sync.dma_start(out=bkt[:], in_=dram["bk"].rearrange("h p -> p h"))
        ones_r = const_p.tile([1, 128], F16, tag="ones_r")
        nc.sync.dma_start(out=ones_r[:], in_=dram["ones_r"][:])
        ones_sq = const_p.tile([128, 128], F16, tag="ones_sq")
        nc.gpsimd.memset(ones_sq[:], 1.0)
        bv_row = const_p.tile([1, ES], F32, tag="bv_row")
        nc.sync.dma_start(out=bv_row[:], in_=dram["bv_row"][:])
        bvb = const_p.tile([128, ES], F32, tag="bvb")

        def _bvb_setup():
            # deferred so this chain never blocks the head of the PE queue
            bvr_r = const_p.tile([1, ES], F16, tag="bvr_r")
            nc.vector.tensor_copy(bvr_r[:], bv_row[:])
            pbv = pp_sc.tile([128, ES], F32, tag="sc", name="pbv")
            nc.tensor.matmul(pbv[:], ones_r[:], bvr_r[:], start=True, stop=True)
            nc.scalar.copy(bvb[:], pbv[:])

        # ---- bulk loads: weights (sync queue), x^T (gpsimd queue, streamed
        # quarter-major so quarter-0 projections start early) ----
        # xt[:, dc*T + t] = xT[dc*128 + p, t]
        xt = big_p.tile([128, DC * T], F16, tag="xt")
        for dc in range(DC):   # quarter-0 pieces: fine-grained, land first
            nc.gpsimd.dma_start(
                out=xt[:, dc * T : dc * T + QW],
                in_=dram["xT"][dc * 128 : (dc + 1) * 128, 0:QW],
            )
        wts = {}
        for name, nsplit in (("wkT", 4), ("wqT", 2), ("wvT", 2)):
            w = big_p.tile([128, DC * ES], F16, tag=name, name=name + "_t")
            step = DC // nsplit
            for s in range(nsplit):
                nc.sync.dma_start(
                    out=w[:].rearrange("p (dc e) -> p dc e", e=ES)
                        [:, s * step : (s + 1) * step],
                    in_=dram[name].rearrange("(dc p) e -> p dc e", p=128)
                        [:, s * step : (s + 1) * step],
                )
            wts[name] = w
        # wot[:, hc*D + eo] = woT[hc*128 + p, eo]
        wot = big_p.tile([128, H_PER * D], F16, tag="wot")
        nc.sync.dma_start(
            out=wot[:].rearrange("p (hc d) -> p hc d", d=D),
            in_=dram["woT"].rearrange("(hc p) d -> p hc d", p=128),
        )
        # x quarters 1-3: one big strided DMA on the sync queue
        nc.sync.dma_start(
            out=xt[:].rearrange("p (dc t) -> p dc t", t=T)[:, :, QW:T],
            in_=dram["xT"].rearrange("(dc p) t -> p dc t", p=128)[:, :, QW:T],
        )

        # ---- projections, one quarter of t at a time (2-bank sub-passes) ----
        # qt/kt[:, h*T + t] : partition = head-dim
        # Vt[:, tc*ES + e]  : partition = t within chunk tc
        qt = big_p.tile([128, H_PER * T], F16, tag="qt")
        kt = big_p.tile([128, H_PER * T], F16, tag="kt")
        Vt = big_p.tile([128, TC * ES], F16, tag="Vt")

        def _kq_subpass(wname, tqa, h):
            w = wts[wname]
            dst, bias = (kt, bkt) if wname == "wkT" else (qt, bqt)
            ps = pp_proj.tile([128, QW], F32, tag="proj", name="ps")
            for dc in range(DC):
                nc.tensor.matmul(
                    ps[:],
                    w[:, dc * ES + h * 128 : dc * ES + (h + 1) * 128],
                    xt[:, dc * T + tqa * QW : dc * T + (tqa + 1) * QW],
                    start=(dc == 0), stop=(dc == DC - 1),
                )
            nc.vector.tensor_scalar_add(
                dst[:, h * T + tqa * QW : h * T + (tqa + 1) * QW],
                ps[:], bias[:, h : h + 1],
            )

        def _v_subpass(tqa, tci):
            wv = wts["wvT"]
            ps = pp_proj.tile([128, ES], F32, tag="proj", name="psv")
            tc_i = 4 * tqa + tci
            for dc in range(DC):
                nc.tensor.matmul(
                    ps[:],
                    xt[:, dc * T + tc_i * 128 : dc * T + tc_i * 128 + 128],
                    wv[:, dc * ES : (dc + 1) * ES],
                    start=(dc == 0), stop=(dc == DC - 1),
                )
            nc.vector.tensor_add(
                Vt[:, tc_i * ES : (tc_i + 1) * ES], ps[:], bvb[:]
            )

        def _proj_quarter_units(tqa):
            # each unit is one 16-matmul sub-pass (single PSUM bank, so the
            # 2-buf pool gives a full sub-pass of WAR reuse distance)
            for h in range(H_PER):
                yield lambda t=tqa, h=h: _kq_subpass("wkT", t, h)
            for h in range(H_PER):
                yield lambda t=tqa, h=h: _kq_subpass("wqT", t, h)
            for tci in range(4):
                yield lambda t=tqa, p=tci: _v_subpass(t, p)

        # ---- out-projection for one quarter (emitted as filler groups) ----
        # 2 PSUM banks: eo-pairs, so each (tci, ep) pass accumulates over hc.
        def _op_group(ctxT, t0, tci, ep, hc, psos):
            st = ctxT[:, hc * QW + tci * 128 : hc * QW + tci * 128 + 128]
            for eo in range(2):
                eoa = 2 * ep + eo
                nc.tensor.matmul(
                    psos[eo][:], st,
                    wot[:, hc * D + eoa * ES : hc * D + (eoa + 1) * ES],
                    start=(hc == 0), stop=(hc == H_PER - 1),
                )
            if hc == H_PER - 1:
                ost = os_p.tile([128, 2 * ES], F16, name="ost", tag=f"ost{ep}")
                for eo in range(2):
                    cp = nc.vector.tensor_copy if eo == 0 else nc.scalar.copy
                    cp(ost[:, eo * ES : (eo + 1) * ES], psos[eo][:])
                dma = nc.sync.dma_start if (tci + ep) % 2 == 0 else nc.gpsimd.dma_start
                dma(
                    out=out_ap[
                        t0 + tci * 128 : t0 + tci * 128 + 128,
                        2 * ep * ES : 2 * (ep + 1) * ES,
                    ],
                    in_=ost[:],
                )

        def _op_units(ctxT, t0):
            for tci in range(4):
                for ep in range(2):
                    psos = [pp_proj.tile([128, ES], F32, tag="proj",
                                         name=f"pso{eo}")
                            for eo in range(2)]
                    for hc in range(H_PER):
                        yield lambda tci=tci, ep=ep, hc=hc, psos=psos: _op_group(
                            ctxT, t0, tci, ep, hc, psos)

        # ---- attention ----
        q0_units = list(_proj_quarter_units(0))
        q0_units[0]()                      # first K sub-pass leads the PE queue
        _bvb_setup()                       # only needed by the V sub-passes
        for u in q0_units[1:]:             # rest of quarter-0 K/Q/V
            u()
        filler = list(_proj_quarter_units(1))

        for qi in range(NQ):
            t0 = qi * QW
            nkc = 4 * qi + 4
            ctxT = ctx_p.tile([128, H_PER * QW], F16)  # [hd, h*QW + t_local]
            n_chunks_left = H_PER * nkc
            for h in range(H_PER):
                pctx = pp_ctx.tile([128, QW], F32, tag="ctx", name="pctx")
                pdn = pp_dn.tile([128, QW], F32, tag="dn", name="pdn")
                ets = {}

                def _score(kc, h=h, ets=ets):
                    # scores for key-chunk kc -> exp -> (mask) -> et
                    j = kc - 4 * qi
                    off = max(0, j * 128)
                    w = QW - off
                    psc = pp_sc.tile([128, QW], F32, tag="sc", name="psc")
                    nc.tensor.matmul(
                        psc[:, :w],
                        kt[:, h * T + kc * 128 : h * T + kc * 128 + 128],
                        qt[:, h * T + t0 + off : h * T + t0 + QW],
                        start=True, stop=True,
                    )
                    et = e_p.tile([128, QW], F16)
                    nc.scalar.activation(et[:, :w], psc[:, :w], EXP)
                    if j >= 0:
                        nc.gpsimd.affine_select(
                            out=et[:, :w], in_=et[:, :w],
                            compare_op=mybir.AluOpType.is_ge,
                            fill=0.0, base=0,
                            pattern=[[1, w]], channel_multiplier=-1,
                        )
                    ets[kc] = et

                def _accum(kc, h=h, pctx=pctx, pdn=pdn, ets=ets):
                    # denominator (broadcast to all 128 partitions via the
                    # all-ones stationary) + AV accumulation for key-chunk kc
                    off = max(0, (kc - 4 * qi) * 128)
                    w = QW - off
                    et = ets.pop(kc)
                    nc.tensor.matmul(
                        pdn[:, off:], ones_sq[:], et[:, :w],
                        start=(kc == 0), stop=(kc == nkc - 1),
                    )
                    nc.tensor.matmul(
                        pctx[:, off:],
                        Vt[:, kc * ES + h * 128 : kc * ES + (h + 1) * 128],
                        et[:, :w],
                        start=(kc == 0), stop=(kc == nkc - 1),
                    )

                # software pipeline: scores run 2 key-chunks ahead of the
                # dependent accumulation matmuls so the PE never waits on the
                # EXP/mask chain; filler (V / out-proj groups) pads the gaps.
                _score(0)
                if nkc > 1:
                    _score(1)
                for kc in range(nkc):
                    if kc + 2 < nkc:
                        _score(kc + 2)
                    n_pop = (len(filler) + n_chunks_left - 1) // n_chunks_left
                    for _ in range(min(n_pop, len(filler))):
                        filler.pop(0)()
                    n_chunks_left -= 1
                    _accum(kc)

                # normalization: pure DVE chain (no PE involvement)
                rb = sm_p.tile([128, QW], F32, tag="rb", name="rb", bufs=2)
                nc.vector.reciprocal_approx_fast(rb[:], pdn[:])
                nc.vector.tensor_mul(
                    ctxT[:, h * QW : (h + 1) * QW], pctx[:], rb[:]
                )
            # drain leftover filler (next quarter's projections must be fully
            # emitted before its attention), then queue the next batch:
            # projections of quarter qi+2 plus this quarter's out-projection.
            for u in filler:
                u()
            filler = []
            if qi + 2 < NQ:
                filler += list(_proj_quarter_units(qi + 2))
            filler += list(_op_units(ctxT, t0))

        for u in filler:
            u()

    nc.compile()
    return nc


def _prepare_in_maps(x, wq, bq, wk, bk, wv, bv, wo, bo):
    s = 1.0 / math.sqrt(HD)
    in_maps = []
    for c in range(N_CORES):
        b = c // 4
        g = c % 4
        es = slice(g * ES, (g + 1) * ES)
        in_maps.append(
            {
                "xT": np.ascontiguousarray(x[b].T).astype(np.float16),
                "wqT": np.ascontiguousarray(wq[es, :].T * s).astype(np.float16),
                "wkT": np.ascontiguousarray(wk[es, :].T).astype(np.float16),
                "wvT": np.ascontiguousarray(wv[es, :].T).astype(np.float16),
                "woT": np.ascontiguousarray(wo[:, es].T).astype(np.float16),
                "bq": (bq[es] * s).astype(np.float32).reshape(H_PER, 128),
                "bk": bk[es].astype(np.float32).reshape(H_PER, 128),
                "bv_row": bv[es].astype(np.float32).reshape(1, ES),
                "ones_r": np.ones((1, 128), np.float16),
            }
        )
    return in_maps


_CACHED_NC = None


def _get_nc():
    global _CACHED_NC
    if _CACHED_NC is None:
        _CACHED_NC = _build()
    return _CACHED_NC


def kernel(x, wq, bq, wk, bk, wv, bv, wo, bo, _trace=False):
    x, wq, bq, wk, bk, wv, bv, wo, bo = (
        np.asarray(a, np.float32) for a in (x, wq, bq, wk, bk, wv, bv, wo, bo)
    )
    nc = _get_nc()
    in_maps = _prepare_in_maps(x, wq, bq, wk, bk, wv, bv, wo, bo)
    res = run_bass_kernel_spmd(nc, in_maps, list(range(N_CORES)), trace=_trace)
    out = np.zeros((B, T, D), np.float32)
    for b in range(B):
        acc = res.results[4 * b]["partial"].astype(np.float32)
        for g in range(1, 4):
            acc = acc + res.results[4 * b + g]["partial"].astype(np.float32)
        out[b] = acc + bo[None, :]
    if _trace:
        return out, res
    return out
